# revision 19
# baseline (speedup 1.0000x reference)
"""Bilateral filter (cv2 semantics: d=9, sigmaColor=sigmaSpace=75, reflect-101
border, inscribed-circle taps, L1 color distance) on 8 Trainium2 NeuronCores.

Contract: kernel(sample=np.float32[8,1024,1024,3]) -> np.float32[8,1024,1024,3].
Data parallel: one image per core.

Algorithm (residual form, symmetric tap pairs): for each pair (t,-t) the
weight field W_t = exp(cc*(L1 color diff)^2 + ln sw) is computed once and the
stacked product G = W_t * (I(.+t) - I(.)) serves both taps:
    acc += G|gather - G|scatter      den += W|gather + W|scatter
    out  = center + acc / den        (center tap contributes d=0, w=1)

Engine split per pair: DVE does the channel-stacked subtract, the two L1
adds, the square and the G multiply; ACT does |.| and exp (the free affine
folds the color coefficient and ln(space weight) into the exp); the
TensorEngine does ALL accumulation as +-identity matmuls into PSUM, which
accumulates in fp32 for free: acc[3,64,16] occupies 6 PSUM banks, den[64,16]
the other 2. Measured (REP-amplified): ~1.23 ms/core; TimelineSim ~1.22 ms.

Layout: all tap shifts are free-dim AP offsets. Each of 128 partitions owns a
[64,16] output block (16 row-bands x 8 col-blocks) with a [3,72,24] channel-
stacked bf16 halo window; 8 column chunks cover the image. Host pre-pads
(reflect), converts to bf16 and extracts halo windows; the device does all
filtering.
"""

import os
import sys

for _p in ("/opt/trn_rl_repo", "/root/.axon_site/_ro/trn_rl_repo"):
    if os.path.isdir(_p) and _p not in sys.path:
        sys.path.insert(0, _p)

import numpy as np
import ml_dtypes

import concourse.bass as bass
import concourse.bacc as bacc
import concourse.mybir as mybir
import concourse.tile as tile
from concourse.bass_utils import run_bass_kernel_spmd

BF16 = ml_dtypes.bfloat16

# Filter constants (must match the reference).
D = 9
R = D // 2  # 4
SIGMA_COLOR = 75.0
SIGMA_SPACE = 75.0
COLOR_COEFF = -0.5 / (SIGMA_COLOR * SIGMA_COLOR)
SPACE_COEFF = -0.5 / (SIGMA_SPACE * SIGMA_SPACE)

B, H, W, C = 8, 1024, 1024, 3
N_CORES = 8

# Device geometry: per chunk, 128 partitions = 16 row-bands x 8 col-blocks,
# each owning a [BR, BC] output block with a [BR+2R, BC+2R] halo window.
BR, BC = 64, 16
EY, EX = BR + 2 * R, BC + 2 * R  # 72, 24
ROW_BANDS = H // BR  # 16
COL_BLOCKS_PER_CHUNK = 128 // ROW_BANDS  # 8
CHUNKS = W // (BC * COL_BLOCKS_PER_CHUNK)  # 8
MM = 512  # one PSUM bank of fp32 = one matmul output
QROWS = MM // BC  # rows per matmul slice (32)
NQ = BR // QROWS  # matmul slices per [BR,BC] plane (2)

# Symmetric tap pairs of the inscribed-circle 9x9 stencil: (dy,dx) with
# dy>0, or dy==0 and dx>0. The center tap is implicit (d=0, w=1).
PAIRS = [
    (dy, dx)
    for dy in range(0, R + 1)
    for dx in range(-R, R + 1)
    if dy * dy + dx * dx <= R * R and (dy > 0 or dx > 0)
]
assert len(PAIRS) == 24


def _space_weight(dy, dx):
    return float(np.exp(SPACE_COEFF * (dy * dy + dx * dx)).astype(np.float32))


def _cbc(ap3, c=C):
    """Broadcast a [P, y, x] AP along a new channel axis -> [P, c, y, x]."""
    return bass.AP(ap3.tensor, ap3.offset, [ap3.ap[0], [0, c]] + list(ap3.ap[1:]))


def _build_nc():
    """Build + compile the per-core Bass program once."""
    nc = bacc.Bacc(None, target_bir_lowering=False)
    # Register const APs for the activation bias values ln(space_weight).
    for _dy, _dx in PAIRS:
        v = float(np.log(_space_weight(_dy, _dx)))
        if (mybir.dt.float32, v) not in nc.const_aps.aps:
            t = nc.alloc_sbuf_tensor(f"const-lnsw-{_dy}-{_dx}", [128, 1], mybir.dt.float32)
            nc.gpsimd.memset(t.ap(), v)
            nc.const_aps.aps[(mybir.dt.float32, v)] = t.ap()
    nc.all_engine_barrier()
    inp = nc.declare_dram_parameter(
        "win", [CHUNKS, 128, C, EY, EX], mybir.dt.bfloat16, isOutput=False
    )
    eye_in = nc.declare_dram_parameter(
        "eye", [128, 256], mybir.dt.bfloat16, isOutput=False
    )
    outp = nc.declare_dram_parameter(
        "out", [CHUNKS, 128, C, BR, BC], mybir.dt.bfloat16, isOutput=True
    )

    bf16 = mybir.dt.bfloat16
    f32 = mybir.dt.float32
    Act = mybir.ActivationFunctionType

    with tile.TileContext(nc) as tc:
        with (
            nc.allow_low_precision(
                "residual-form bf16 pipeline: out = center + acc/den where "
                "acc/den are small corrections accumulated in fp32 PSUM; "
                "validated rel-err ~2e-3"
            ),
            tc.tile_pool(name="singles", bufs=1) as singles,
            tc.tile_pool(name="img", bufs=3) as img_pool,
            tc.tile_pool(name="dpool", bufs=3) as d_pool,
            tc.tile_pool(name="wpip", bufs=3) as w_pool,
            tc.tile_pool(name="gpool", bufs=3) as g_pool,
            tc.tile_pool(name="smallp", bufs=2) as small_pool,
            tc.tile_pool(name="outp", bufs=2) as out_pool,
            tc.tile_pool(name="psum", bufs=1, space="PSUM") as psum_pool,
        ):
          eye = singles.tile([128, 256], bf16, tag="eye", name="eye")
          nc.sync.dma_start(eye[:], eye_in[:])
          eyeP = eye[:, 0:128]
          eyeN = eye[:, 128:256]

          for _rep in range(int(os.environ.get("BILAT_REP", "1"))):
            for ch in range(CHUNKS):
                I = img_pool.tile([128, C, EY, EX], bf16, tag="I", name="I")
                nc.sync.dma_start(I[:], inp[ch])

                acc = psum_pool.tile([128, C, BR, BC], f32, tag="acc", name="acc")
                den = psum_pool.tile([128, BR, BC], f32, tag="den", name="den")

                for ip, (dy, dx) in enumerate(PAIRS):
                    first = ip == 0
                    last = ip == len(PAIRS) - 1
                    # Weight-field region R_t (tile coords): origin (ry,cx),
                    # size (sy,sx). Covers output pixels and output-minus-t.
                    ry = R - dy
                    cx = R - max(dx, 0)
                    sy = BR + dy
                    sx = BC + abs(dx)

                    dt_ = d_pool.tile([128, C, EY, EX], bf16, tag="d", name="d")
                    ab = w_pool.tile([128, C, EY, EX], bf16, tag="ab", name="ab")
                    s_a = w_pool.tile([128, EY, EX], bf16, tag="s_a", name="s_a")
                    s_b = w_pool.tile([128, EY, EX], bf16, tag="s_b", name="s_b")
                    Wt = w_pool.tile([128, EY, EX], bf16, tag="Wt", name="Wt")

                    # d = I(.+t) - I(.) on R_t (all channels), stored at origin 0.
                    nc.vector.tensor_sub(
                        dt_[:, :, :sy, :sx],
                        I[:, :, ry + dy : ry + dy + sy, cx + dx : cx + dx + sx],
                        I[:, :, ry : ry + sy, cx : cx + sx],
                    )
                    # s = |d0| + |d1| + |d2|  (abs on ACT, adds on DVE)
                    nc.scalar.activation(
                        ab[:, :, :sy, :sx], dt_[:, :, :sy, :sx], Act.Abs
                    )
                    nc.vector.tensor_add(
                        s_a[:, :sy, :sx], ab[:, 0, :sy, :sx], ab[:, 1, :sy, :sx]
                    )
                    nc.vector.tensor_add(
                        s_b[:, :sy, :sx], s_a[:, :sy, :sx], ab[:, 2, :sy, :sx]
                    )
                    # W = exp(color_coeff * s^2 + ln(space_weight))
                    nc.vector.tensor_mul(
                        s_a[:, :sy, :sx], s_b[:, :sy, :sx], s_b[:, :sy, :sx]
                    )
                    nc.scalar.activation(
                        Wt[:, :sy, :sx],
                        s_a[:, :sy, :sx],
                        Act.Exp,
                        bias=float(np.log(_space_weight(dy, dx))),
                        scale=COLOR_COEFF,
                    )

                    G = g_pool.tile([128, C, EY, EX], bf16, tag="G", name="G")
                    nc.vector.tensor_mul(
                        G[:, :, :sy, :sx],
                        _cbc(Wt[:, :sy, :sx]),
                        dt_[:, :, :sy, :sx],
                    )

                    # Accumulate on PE: gather slice at origin (dy, gx),
                    # scatter slice at origin (0, sx0), within R_t coords.
                    gy, gx = dy, max(dx, 0)
                    sy0, sx0 = 0, max(-dx, 0)
                    # +I group: acc += G|gather, den += W|gather + W|scatter
                    for c in range(C):
                        for q in range(NQ):
                            r0 = gy + q * QROWS
                            nc.tensor.matmul(
                                acc[:, c, q * QROWS : (q + 1) * QROWS, :],
                                eyeP,
                                G[:, c, r0 : r0 + QROWS, gx : gx + BC],
                                start=first,
                                stop=False,
                            )
                    for q in range(NQ):
                        nc.tensor.matmul(
                            den[:, q * QROWS : (q + 1) * QROWS, :],
                            eyeP,
                            Wt[:, gy + q * QROWS : gy + (q + 1) * QROWS, gx : gx + BC],
                            start=first,
                            stop=False,
                        )
                        nc.tensor.matmul(
                            den[:, q * QROWS : (q + 1) * QROWS, :],
                            eyeP,
                            Wt[:, sy0 + q * QROWS : sy0 + (q + 1) * QROWS, sx0 : sx0 + BC],
                            start=False,
                            stop=last,
                        )
                    # -I group: acc -= G|scatter
                    for c in range(C):
                        for q in range(NQ):
                            r0 = sy0 + q * QROWS
                            nc.tensor.matmul(
                                acc[:, c, q * QROWS : (q + 1) * QROWS, :],
                                eyeN,
                                G[:, c, r0 : r0 + QROWS, sx0 : sx0 + BC],
                                start=False,
                                stop=last,
                            )

                # Epilogue: out = center + acc / den   (acc, den in fp32 PSUM)
                den1 = small_pool.tile([128, BR, BC], f32, tag="den1", name="den1")
                nc.vector.tensor_scalar_add(den1[:], den[:], 1.0)  # center tap w=1
                rec = small_pool.tile([128, BR, BC], f32, tag="rec", name="rec")
                nc.vector.reciprocal(rec[:], den1[:])
                macc = small_pool.tile([128, C, BR, BC], bf16, tag="macc", name="macc")
                nc.vector.tensor_mul(macc[:], acc[:], _cbc(rec[:]))
                ot = out_pool.tile([128, C, BR, BC], bf16, tag="ot", name="ot")
                nc.vector.tensor_add(
                    ot[:], macc[:], I[:, :, R : R + BR, R : R + BC]
                )
                nc.sync.dma_start(outp[ch], ot[:])

    nc.compile()
    return nc


_NC_CACHE = {}


def _get_nc():
    if "nc" not in _NC_CACHE:
        _NC_CACHE["nc"] = _build_nc()
    return _NC_CACHE["nc"]


def _eye_input():
    e = np.zeros((128, 256), dtype=np.float32)
    e[:, :128] = np.eye(128, dtype=np.float32)
    e[:, 128:] = -np.eye(128, dtype=np.float32)
    return e.astype(BF16)


def _prep_core_input(img):
    """[H,W,C] f32 -> [CHUNKS, 128, C, EY, EX] bf16 halo windows."""
    padded = np.pad(img, ((R, R), (R, R), (0, 0)), mode="reflect")
    padded = np.ascontiguousarray(padded.transpose(2, 0, 1)).astype(BF16)  # [C,1032,1032]
    sw = np.lib.stride_tricks.sliding_window_view(padded, (EY, EX), axis=(1, 2))
    # sw[c, y0, x0] = padded[c, y0:y0+EY, x0:x0+EX]
    wins = sw[:, :: BR, :: BC]  # [C, ROW_BANDS, W//BC, EY, EX]
    wins = wins.reshape(C, ROW_BANDS, CHUNKS, COL_BLOCKS_PER_CHUNK, EY, EX)
    wins = wins.transpose(2, 1, 3, 0, 4, 5)  # [CHUNKS, 16, 8, C, EY, EX]
    return np.ascontiguousarray(wins).reshape(CHUNKS, 128, C, EY, EX)


def _assemble_core_output(out):
    """[CHUNKS, 128, C, BR, BC] bf16 -> [H,W,C] f32."""
    o = out.reshape(CHUNKS, ROW_BANDS, COL_BLOCKS_PER_CHUNK, C, BR, BC)
    o = o.transpose(3, 1, 4, 0, 2, 5)  # [C, 16, BR, CHUNKS, 8, BC]
    o = o.reshape(C, H, W).transpose(1, 2, 0)
    return np.ascontiguousarray(o, dtype=np.float32)


def kernel(sample):
    sample = np.asarray(sample, dtype=np.float32)
    assert sample.shape == (B, H, W, C)
    nc = _get_nc()
    eye = _eye_input()
    in_maps = [
        {"win": _prep_core_input(sample[i]), "eye": eye} for i in range(B)
    ]
    res = run_bass_kernel_spmd(nc, in_maps, list(range(N_CORES)))
    return np.stack(
        [_assemble_core_output(res.results[i]["out"]) for i in range(B)], axis=0
    )


if __name__ == "__main__":
    x = np.random.RandomState(0).rand(B, H, W, C).astype(np.float32) * 255.0
    y = kernel(x)
    print("kernel output:", y.shape, y.dtype, float(y.min()), float(y.max()))
